# revision 1
# baseline (speedup 1.0000x reference)
"""Trainium2 Bass kernel for nn_KANNetwork (3-layer KAN + linear skip).

Sharding: data-parallel over batch (16384/8 = 2048 rows per core); coeffs
replicated. Batch statistics (mean/std over the full batch) via a tiny
per-layer AllReduce of [sum, sumsq] partial sums.

Layout: everything feature-major [feat, batch] on-chip. Each layer's matmul
(out-features on PSUM partitions, batch on the free axis) directly produces
the transposed input of the next layer, so only x is transposed (on the PE).

Gaussian-basis trick: with centers c_k = -2 + h*k (h = 4/15),
  basis_k = exp(-2(xn - c_k)^2) = e1 * prod_{j<=k} m_j
  e1  = exp(-2 (xn + 2)^2)                (= basis_0 exactly)
  m_1 = exp(8h - 2h^2) * t,  t = exp((16/15) xn),  m_k = exp(-4h^2) * m_{k-1}
so each of the 16 basis functions costs one DVE multiply + one ACT
scalar-multiply instead of an exp — the ScalarE does 3 exps total.

Matmuls run in float32r (operands rounded to ~11 mantissa bits, full-rate on
the PE at N>=256; one layer is ~1.6e-4 relative error).
"""
import numpy as np
import bass_rust
import concourse.bass as bass
import concourse.tile as tile
from concourse import mybir
from concourse.bass_utils import run_bass_kernel_spmd

F32 = mybir.dt.float32
F32R = mybir.dt.float32r
AF = mybir.ActivationFunctionType
ALU = mybir.AluOpType

NCORES = 8
P = 128
B_FULL = 16384
BS = B_FULL // NCORES        # 2048 rows per core
BCH = 512                    # batch chunk (one PSUM bank of fp32)
NBCH = BS // BCH             # 4
NK = 16                      # basis functions
IN_F = 256
HID = 512

H_STEP = 4.0 / 15.0
T_SCALE = 16.0 / 15.0                              # t = exp(T_SCALE * xn)
M1_SCALE = float(np.exp(8 * H_STEP - 2 * H_STEP * H_STEP))
W_RATIO = float(np.exp(-4 * H_STEP * H_STEP))
MK_CONST = {k: M1_SCALE * W_RATIO ** (k - 1) for k in range(1, NK)}
DIRECT_KS = (2, 5, 8, 11, 14)          # ACT-computed anchors; others chain on DVE
CENTERS = [-2.0 + H_STEP * k for k in range(NK)]


def split_multi_waits(nc):
    """This walrus build accepts one sem-wait per instruction; hoist extras
    onto standalone NoOps on the same engine stream (in-order => safe)."""
    n = 0
    for bb in nc.main_func.blocks:
        out = []
        for inst in bb.instructions:
            si = inst.sync_info
            if si is not None and si.on_wait is not None and len(si.on_wait) > 1:
                ws = list(si.on_wait)
                for w in ws[:-1]:
                    n += 1
                    nop = bass_rust.InstNoOp(name=f"I-wsplit-{n}")
                    nop.engine = inst.engine
                    nop.sync_info = mybir.SyncInfo(on_wait=[w], on_update=[])
                    out.append(nop)
                inst.sync_info = mybir.SyncInfo(
                    on_wait=[ws[-1]], on_update=list(si.on_update)
                )
            out.append(inst)
        bb.instructions = out
    return n


def _stats_to_norm(nc, pools, sums, ssq, nf_ch):
    """From global [sum, sumsq] per feature -> per-partition scale/bias tiles
    rsd (1/(sd+1e-6)) and nb (-mu*rsd), each [128, nf_ch]."""
    small = pools["small"]
    mu = small.tile([P, nf_ch], F32, tag="mu")
    t1 = small.tile([P, nf_ch], F32, tag="t1")
    var = small.tile([P, nf_ch], F32, tag="var")
    sd = small.tile([P, nf_ch], F32, tag="sd")
    rsd = small.tile([P, nf_ch], F32, tag=f"rsd{nf_ch}_{pools['uid'][0]}")
    nb = small.tile([P, nf_ch], F32, tag=f"nb{nf_ch}_{pools['uid'][0]}")
    pools["uid"][0] += 1
    nc.vector.tensor_scalar(out=mu, in0=sums, scalar1=1.0 / B_FULL, scalar2=None,
                            op0=ALU.mult)
    nc.vector.tensor_mul(t1, mu, sums)                      # sum^2/B
    nc.vector.tensor_sub(var, ssq, t1)                      # (B-1)*var
    nc.scalar.activation(out=sd, in_=var, func=AF.Sqrt,
                         scale=1.0 / (B_FULL - 1))          # sd
    # one Newton polish for the (loosely-toleranced) ACT sqrt:
    # sd' = 0.5*(sd + var/( (B-1) sd ))
    rc = small.tile([P, nf_ch], F32, tag="rc")
    nc.vector.reciprocal(rc, sd)
    nc.vector.tensor_scalar(out=t1, in0=var, scalar1=1.0 / (B_FULL - 1),
                            scalar2=None, op0=ALU.mult)
    nc.vector.tensor_mul(t1, t1, rc)                        # var/sd
    nc.vector.tensor_add(sd, sd, t1)
    nc.vector.tensor_scalar(out=sd, in0=sd, scalar1=0.5, scalar2=1e-6,
                            op0=ALU.mult, op1=ALU.add)      # sd + 1e-6
    nc.vector.reciprocal(rsd, sd)
    nc.vector.tensor_mul(nb, mu, rsd)
    nc.vector.tensor_scalar(out=nb, in0=nb, scalar1=-1.0, scalar2=None,
                            op0=ALU.mult)
    return rsd, nb


def _allreduce_stats(nc, pools, sums_t, ssq_t, nf_ch, tag):
    """DMA [sums|ssq] ([128, nf_ch] each) to DRAM, AllReduce, load back."""
    dram = pools["dram"]
    small = pools["small"]
    cin = dram.tile([P, 2 * nf_ch], F32, tag=f"cin{tag}")
    cout = dram.tile([P, 2 * nf_ch], F32, tag=f"cout{tag}")
    nc.sync.dma_start(out=cin[:, 0:nf_ch], in_=sums_t)
    nc.sync.dma_start(out=cin[:, nf_ch:2 * nf_ch], in_=ssq_t)
    nc.gpsimd.collective_compute(
        "AllReduce", ALU.add,
        replica_groups=[list(range(NCORES))],
        ins=[cin.opt()], outs=[cout.opt()],
    )
    gl = small.tile([P, 2 * nf_ch], F32, tag=f"gl{tag}")
    nc.sync.dma_start(out=gl, in_=cout)
    return gl[:, 0:nf_ch], gl[:, nf_ch:2 * nf_ch]


def build_program():
    nc = bass.Bass("TRN2", target_bir_lowering=False, debug=False,
                   num_devices=NCORES)

    x_d = nc.dram_tensor("x", [BS, IN_F], F32, kind="ExternalInput")
    c1_d = nc.dram_tensor("c1t", [NK, IN_F, HID], F32R, kind="ExternalInput")
    c2_d = nc.dram_tensor("c2t", [NK, HID, HID], F32R, kind="ExternalInput")
    c3_d = nc.dram_tensor("c3t", [NK, HID, 1], F32R, kind="ExternalInput")
    skw_d = nc.dram_tensor("skwt", [IN_F, 1], F32R, kind="ExternalInput")
    skb_d = nc.dram_tensor("skb", [1, 1], F32, kind="ExternalInput")
    out_d = nc.dram_tensor("out", [1, BS], F32, kind="ExternalOutput")

    ident_d = nc.inline_tensor(np.eye(P, dtype=np.float32), name="ident")

    with tile.TileContext(nc) as tc:
        import contextlib
        ctx = contextlib.ExitStack()
        with ctx:
            persist = ctx.enter_context(tc.tile_pool(name="persist", bufs=1))
            small = ctx.enter_context(tc.tile_pool(name="small", bufs=2))
            dram = ctx.enter_context(tc.tile_pool(name="dram", bufs=1, space="DRAM"))
            cpool = ctx.enter_context(tc.tile_pool(name="cstream", bufs=4))
            bpool = ctx.enter_context(tc.tile_pool(name="basis", bufs=4))
            xpool = ctx.enter_context(tc.tile_pool(name="xn", bufs=1))
            spool = ctx.enter_context(tc.tile_pool(name="setup", bufs=1))
            scrap = ctx.enter_context(tc.tile_pool(name="scrap", bufs=1))
            xload = ctx.enter_context(tc.tile_pool(name="xload", bufs=4))
            pmm = ctx.enter_context(tc.tile_pool(name="pmm", bufs=1, space="PSUM"))
            pmisc = ctx.enter_context(tc.tile_pool(name="pmisc", bufs=1, space="PSUM"))
            pl3 = ctx.enter_context(tc.tile_pool(name="pl3", bufs=1, space="PSUM"))

            pools = {"small": small, "dram": dram, "uid": [0]}

            # ---- constants / tiny inputs ----
            ident = persist.tile([P, P], F32, tag="ident")
            nc.sync.dma_start(out=ident, in_=ident_d[:, :])
            skw = persist.tile([P, 2], F32R, tag="skw")
            nc.sync.dma_start(out=skw, in_=skw_d.ap().rearrange("(ic p) o -> p (ic o)", p=P))
            skb = persist.tile([1, 1], F32, tag="skb")
            nc.sync.dma_start(out=skb, in_=skb_d[:, :])
            two_c = persist.tile([P, 1], F32, tag="two_c")
            nc.vector.memset(two_c, 2.0)
            negck = {}
            for k in DIRECT_KS:
                ck = persist.tile([P, 1], F32, tag=f"negc{k}", name=f"negc{k}")
                nc.vector.memset(ck, -CENTERS[k])
                negck[k] = ck
            c3sb = persist.tile([P, NK, 4], F32R, tag="c3sb")
            nc.sync.dma_start(out=c3sb, in_=c3_d.ap().rearrange("k (ic p) o -> p k (ic o)", p=P))

            # ---- transpose x into xT [128, 2, 2048] (+ fp32r copy) ----
            xT = persist.tile([P, 2, BS], F32, tag="xT")
            for ib in range(BS // P):          # 16 batch tiles
                xnat = xload.tile([P, IN_F], F32, tag="xnat")
                nc.sync.dma_start(out=xnat, in_=x_d[ib * P:(ib + 1) * P, :])
                for ic in range(2):
                    pt = pmisc.tile([P, P], F32, tag="tr")
                    nc.tensor.transpose(pt[:, :], xnat[:, ic * P:(ic + 1) * P], ident[:, :])
                    nc.vector.tensor_copy(xT[:, ic, ib * P:(ib + 1) * P], pt[:, :])
            # ---- layer-1 stats of x ----
            sums1 = small.tile([P, 2], F32, tag="sums1")
            ssq1 = small.tile([P, 2], F32, tag="ssq1")
            nc.vector.tensor_reduce(out=sums1, in_=xT, axis=mybir.AxisListType.X,
                                    op=ALU.add)
            ssq1p = small.tile([P, 2, NBCH], F32, tag="ssq1p")
            for ic in range(2):
                for bq in range(NBCH):
                    sc = scrap.tile([P, BCH], F32, tag="sq_scrap")
                    nc.scalar.activation(
                        out=sc, in_=xT[:, ic, bq * BCH:(bq + 1) * BCH],
                        func=AF.Square, accum_out=ssq1p[:, ic, bq:bq + 1])
            nc.vector.tensor_reduce(out=ssq1, in_=ssq1p,
                                    axis=mybir.AxisListType.X, op=ALU.add)
            gs, gq = _allreduce_stats(nc, pools, sums1, ssq1, 2, "l1")
            rsd1, nb1 = _stats_to_norm(nc, pools, gs, gq, 2)

            skip_sb = persist.tile([1, BS], F32, tag="skip_sb")

            def kan_layer(h_in, h_in_r, nf_ch, no_ch, c_dram, h_out, rsd, nb,
                          sums_n, ssq_n, layer):
                """One KAN layer in feature-major layout.
                h_in: [128, nf_ch, BS] (F32) normalized input source
                h_out: [128, no_ch, BS] or None (layer 3 -> scalar path)
                """
                last = layer == 3
                for bc in range(NBCH):
                    bsl = slice(bc * BCH, (bc + 1) * BCH)
                    # normalize + clip
                    xnr = xpool.tile([P, nf_ch, BCH], F32, tag="xnr", padded_shape=[P, 4, BCH])
                    xn = xpool.tile([P, nf_ch, BCH], F32, tag="xn", padded_shape=[P, 4, BCH])
                    for ic in range(nf_ch):
                        nc.scalar.activation(out=xnr[:, ic, :], in_=h_in[:, ic, bsl],
                                             func=AF.Identity,
                                             scale=rsd[:, ic:ic + 1],
                                             bias=nb[:, ic:ic + 1])
                    nc.vector.tensor_scalar(out=xn, in0=xnr, scalar1=3.0,
                                            scalar2=-3.0, op0=ALU.min, op1=ALU.max)
                    # basis setup: s=(xn+2)^2, e1=exp(-2s)=basis_0, t=exp(g*xn)
                    s = spool.tile([P, nf_ch, BCH], F32, tag="s", padded_shape=[P, 4, BCH])
                    t = spool.tile([P, nf_ch, BCH], F32, tag="t", padded_shape=[P, 4, BCH])
                    nc.scalar.activation(out=s, in_=xn, func=AF.Square, bias=two_c[:, 0:1])
                    basis = bpool.tile([P, nf_ch, BCH], F32R, tag="b", padded_shape=[P, 4, BCH])
                    nc.scalar.activation(out=basis, in_=s, func=AF.Exp, scale=-2.0)
                    nc.scalar.activation(out=t, in_=xn, func=AF.Exp, scale=T_SCALE)

                    if last:
                        ps_l3 = pl3.tile([1, BCH], F32, tag="pout")
                    else:
                        ps = [pmm.tile([P, BCH], F32, tag=f"pmm{oc}", name=f"pmm{oc}",
                                     bufs=(2 if oc < 2 else 1))
                              for oc in range(no_ch)]
                    for k in range(NK):
                        if k in DIRECT_KS:
                            # independent ACT anchor: basis_k = exp(-2(xn-c_k)^2)
                            sk = spool.tile([P, nf_ch, BCH], F32, tag="s",
                                            padded_shape=[P, 4, BCH], name="sk")
                            nc.scalar.activation(out=sk, in_=xn, func=AF.Square,
                                                 bias=negck[k][:, 0:1])
                            basis_new = bpool.tile([P, nf_ch, BCH], F32R,
                                                   tag="b", padded_shape=[P, 4, BCH])
                            nc.scalar.activation(out=basis_new, in_=sk,
                                                 func=AF.Exp, scale=-2.0)
                            basis = basis_new
                        elif k > 0:
                            # basis_k = (t * mk) * basis_{k-1} in one DVE op
                            basis_new = bpool.tile([P, nf_ch, BCH], F32R,
                                                   tag="b", padded_shape=[P, 4, BCH])
                            nc.vector.scalar_tensor_tensor(
                                out=basis_new, in0=t, scalar=MK_CONST[k],
                                in1=basis, op0=ALU.mult, op1=ALU.mult)
                            basis = basis_new
                        if last:
                            for ic in range(nf_ch):
                                nc.tensor.matmul(
                                    ps_l3[:, :], c3sb[:, k, ic:ic + 1],
                                    basis[:, ic, :],
                                    start=(k == 0 and ic == 0),
                                    stop=(k == NK - 1 and ic == nf_ch - 1),
                                )
                        else:
                            ctile = cpool.tile([P, nf_ch, HID], F32R,
                                               tag="c", padded_shape=[P, 4, HID])
                            dma_eng = nc.sync if (k % 2 == 0) else nc.gpsimd
                            dma_eng.dma_start(
                                out=ctile,
                                in_=c_dram[k].rearrange("(ic p) o -> p ic o", p=P))
                            for ic in range(nf_ch):
                                for oc in range(no_ch):
                                    nc.tensor.matmul(
                                        ps[oc][:, :],
                                        ctile[:, ic, oc * P:(oc + 1) * P],
                                        basis[:, ic, :],
                                        start=(k == 0 and ic == 0),
                                        stop=(k == NK - 1 and ic == nf_ch - 1),
                                    )
                    if last:
                        # add the precomputed skip row and write output chunk
                        nc.vector.tensor_add(out_sb[:, bsl], ps_l3[:, :],
                                             skip_sb[:, bsl])
                    else:
                        for oc in range(no_ch):
                            nc.scalar.activation(
                                out=h_out[:, oc, bsl], in_=ps[oc][:, :],
                                func=AF.Tanh,
                                accum_out=sums_n[:, oc, bc:bc + 1])
                            sc = scrap.tile([P, BCH], F32, tag="sq_scrap2")
                            nc.scalar.activation(
                                out=sc, in_=h_out[:, oc, bsl], func=AF.Square,
                                accum_out=ssq_n[:, oc, bc:bc + 1])
                    if layer == 1:
                        # skip path: x @ skip_w (contract 256 feats)
                        xtr = scrap.tile([P, 2, BCH], F32R, tag="xtr")
                        for ic in range(2):
                            nc.vector.tensor_scalar(
                                out=xtr[:, ic, :], in0=h_in[:, ic, bsl],
                                scalar1=1.0, scalar2=None, op0=ALU.mult)
                        ps_sk = pl3.tile([1, BCH], F32, tag="pout", name="ps_sk")
                        for ic in range(2):
                            nc.tensor.matmul(ps_sk[:, :], skw[:, ic:ic + 1],
                                             xtr[:, ic, :],
                                             start=(ic == 0), stop=(ic == 1))
                        nc.scalar.activation(out=skip_sb[:, bsl], in_=ps_sk[:, :],
                                             func=AF.Identity, bias=skb[0:1, 0:1])

            # ---- layer 1 ----
            h1 = persist.tile([P, 4, BS], F32, tag="h1")
            sums2 = small.tile([P, 4, NBCH], F32, tag="sums2")
            ssq2 = small.tile([P, 4, NBCH], F32, tag="ssq2")
            kan_layer(xT, None, 2, 4, c1_d, h1, rsd1, nb1, sums2, ssq2, 1)
            s2 = small.tile([P, 4], F32, tag="s2r")
            q2 = small.tile([P, 4], F32, tag="q2r")
            nc.vector.tensor_reduce(out=s2, in_=sums2, axis=mybir.AxisListType.X, op=ALU.add)
            nc.vector.tensor_reduce(out=q2, in_=ssq2, axis=mybir.AxisListType.X, op=ALU.add)
            gs2, gq2 = _allreduce_stats(nc, pools, s2, q2, 4, "l2")
            rsd2, nb2 = _stats_to_norm(nc, pools, gs2, gq2, 4)

            # ---- layer 2 ----
            h2 = persist.tile([P, 4, BS], F32, tag="h2")
            sums3 = small.tile([P, 4, NBCH], F32, tag="sums3")
            ssq3 = small.tile([P, 4, NBCH], F32, tag="ssq3")
            kan_layer(h1, None, 4, 4, c2_d, h2, rsd2, nb2, sums3, ssq3, 2)
            s3 = small.tile([P, 4], F32, tag="s3r")
            q3 = small.tile([P, 4], F32, tag="q3r")
            nc.vector.tensor_reduce(out=s3, in_=sums3, axis=mybir.AxisListType.X, op=ALU.add)
            nc.vector.tensor_reduce(out=q3, in_=ssq3, axis=mybir.AxisListType.X, op=ALU.add)
            gs3, gq3 = _allreduce_stats(nc, pools, s3, q3, 4, "l3")
            rsd3, nb3 = _stats_to_norm(nc, pools, gs3, gq3, 4)

            # ---- layer 3 + skip ----
            out_sb = persist.tile([1, BS], F32, tag="out_sb")
            kan_layer(h2, None, 4, 1, None, None, rsd3, nb3, None, None, 3)

            nc.sync.dma_start(out=out_d[:, :], in_=out_sb[:, :])

    split_multi_waits(nc)
    return nc


_NC_CACHE = None


def _get_nc():
    global _NC_CACHE
    if _NC_CACHE is None:
        _NC_CACHE = build_program()
    return _NC_CACHE


def kernel(x, coeffs1, coeffs2, coeffs3, skip_w, skip_b, _trace=False):
    x = np.ascontiguousarray(np.asarray(x, np.float32))
    c1t = np.ascontiguousarray(np.transpose(np.asarray(coeffs1, np.float32), (2, 1, 0)))
    c2t = np.ascontiguousarray(np.transpose(np.asarray(coeffs2, np.float32), (2, 1, 0)))
    c3t = np.ascontiguousarray(np.transpose(np.asarray(coeffs3, np.float32), (2, 1, 0)))
    skwt = np.ascontiguousarray(np.asarray(skip_w, np.float32).reshape(1, IN_F).T)
    skb = np.asarray(skip_b, np.float32).reshape(1, 1)

    nc = _get_nc()
    in_maps = [
        {"x": x[i * BS:(i + 1) * BS], "c1t": c1t, "c2t": c2t, "c3t": c3t,
         "skwt": skwt, "skb": skb}
        for i in range(NCORES)
    ]
    res = run_bass_kernel_spmd(nc, in_maps, core_ids=list(range(NCORES)),
                               trace=_trace)
    out = np.concatenate([res.results[i]["out"].reshape(BS) for i in range(NCORES)])
    if _trace:
        return out, res
    return out

